# revision 7
# baseline (speedup 1.0000x reference)
"""ExemplarAttention Trainium2 kernel (8 NeuronCores, batch-sharded).

logits[b,c] = gamma * log(sum_{n:label[n]=c} exp(-beta * sum_k w_k (x[b,k]-e[n,k])^2) + eps)

Strategy:
  - Data-parallel over batch B=1024 across 8 cores (128 rows/core = one partition tile).
  - Host precomputes the tiny constrained params (softmax(w), beta, gamma),
    x^2@w (per-row bias), e^2@w, and sorts exemplars by class label so the
    per-class scatter-add becomes contiguous segment sums.
  - On device, per 2048-column PSUM super-tile:
      psum = (ones x -e2w/2) [K=1 matmul, start=True]
           + sum_kc xw_t[kc].T @ e_t[kc]   [4 K=128 matmuls]
    i.e. psum[m,n] = cross[m,n] - e2w[n]/2.
  - ScalarE: exp(2*beta*psum + (-beta*x2w)[m]) per class-segment piece with
    accum_out -> per-class partial sums directly (no one-hot GEMM, no transpose).
  - Tail: one 3D tensor_reduce merges the piece partials, Ln(+1e-9), *gamma, DMA out.
"""

import os
from contextlib import ExitStack

import numpy as np

B, N, D, C = 1024, 16384, 512, 10
NCORES = 8
B_LOC = B // NCORES          # 128
KC = D // 128                # 4 contraction chunks
SUPER = 2048                 # psum super-tile width (4 banks)
NSUPER = N // SUPER
NTILE = 512                  # matmul free dim (1 psum bank)
EPS = 1e-9

# e_t dtype: "bfloat16" or "float8e4".  fp8 halves the dominant DMA stream.
E_DTYPE = os.environ.get("BASSK_E_DTYPE", "bfloat16")

_prog_cache = {}


def _np_dt(mybir, name):
    return mybir.dt.np(getattr(mybir.dt, name))


def _compute_pieces(counts):
    """Split each class's sorted-exemplar segment at SUPER boundaries.

    Returns (pieces, maxp): pieces is a list of (super_idx, cls, piece_idx,
    g0, g1) with global column range [g0, g1)."""
    starts = np.concatenate([[0], np.cumsum(counts)]).astype(int)
    pieces = []
    piece_counter = [0] * C
    for c in range(C):
        g0, g1 = int(starts[c]), int(starts[c + 1])
        while g0 < g1:
            end = min(g1, (g0 // SUPER + 1) * SUPER)
            pieces.append((g0 // SUPER, c, piece_counter[c], g0, end))
            piece_counter[c] += 1
            g0 = end
    maxp = max(piece_counter) if max(piece_counter) > 0 else 1
    return pieces, maxp


def _build_program(pieces, maxp, beta, gamma):
    import concourse.bass as bass  # noqa: F401
    import concourse.tile as tile
    from concourse import bacc, mybir

    e_dt = getattr(mybir.dt, E_DTYPE)
    bf16 = mybir.dt.bfloat16
    f32 = mybir.dt.float32

    nc = bacc.Bacc("TRN2", target_bir_lowering=False, debug=False,
                   num_devices=NCORES)

    e_t_d = nc.dram_tensor("e_t", [KC, 128, N], e_dt, kind="ExternalInput").ap()
    xw_t_d = nc.dram_tensor("xw_t", [128, KC, B_LOC], bf16, kind="ExternalInput").ap()
    aug_d = nc.dram_tensor("aug", [1, N + 128], bf16, kind="ExternalInput").ap()
    bias_d = nc.dram_tensor("bias", [B_LOC, 1], f32, kind="ExternalInput").ap()
    out_d = nc.dram_tensor("logits", [B_LOC, C], f32, kind="ExternalOutput").ap()

    two_beta = float(2.0 * beta)

    by_super = [[] for _ in range(NSUPER)]
    for s, c, p, g0, g1 in pieces:
        by_super[s].append((c, p, g0, g1))

    with tile.TileContext(nc) as tc, ExitStack() as ctx:
        singles = ctx.enter_context(tc.tile_pool(name="singles", bufs=1))
        et_pool = ctx.enter_context(tc.tile_pool(name="et", bufs=3 * KC))
        psum_pool = ctx.enter_context(tc.tile_pool(name="ps", bufs=2, space="PSUM"))
        sc_pool = ctx.enter_context(tc.tile_pool(name="sc", bufs=2))

        xw_sb = singles.tile([128, KC, B_LOC], bf16)
        nc.sync.dma_start(out=xw_sb[:, :, :], in_=xw_t_d[:, :, :])
        # aug row (-e2w/2) and the ones row for the K=1 matmul share one tile
        # so their base partitions match.
        aug_sb = singles.tile([1, N + 128], bf16)
        nc.sync.dma_start(out=aug_sb[:, :], in_=aug_d[:, :])
        bias_sb = singles.tile([B_LOC, 1], f32)
        nc.sync.dma_start(out=bias_sb[:, :], in_=bias_d[:, :])

        acc = singles.tile([128, C * maxp], f32)
        nc.vector.memset(acc[:, :], 0.0)
        eps_sb = singles.tile([128, 1], f32)
        nc.vector.memset(eps_sb[:, :], float(EPS))

        for s in range(NSUPER):
            ets = []
            for kc in range(KC):
                et = et_pool.tile([128, SUPER], e_dt, tag="et")
                nc.sync.dma_start(
                    out=et[:, :],
                    in_=e_t_d[kc, :, s * SUPER:(s + 1) * SUPER],
                )
                ets.append(et)

            ps = psum_pool.tile([128, SUPER], f32)
            for j in range(SUPER // NTILE):
                cs = slice(j * NTILE, (j + 1) * NTILE)
                gcs = slice(s * SUPER + j * NTILE, s * SUPER + (j + 1) * NTILE)
                # psum = ones.T @ (-e2w/2)  (K=1, fills whole tile, clears bank)
                nc.tensor.matmul(ps[:, cs], lhsT=aug_sb[:, N:N + B_LOC],
                                 rhs=aug_sb[:, gcs], start=True, stop=False)
                for kc in range(KC):
                    nc.tensor.matmul(ps[:, cs], lhsT=xw_sb[:, kc, :],
                                     rhs=ets[kc][:, cs], start=False,
                                     stop=(kc == KC - 1))

            sc = sc_pool.tile([128, SUPER], f32, tag="sc")
            for c, p, g0, g1 in by_super[s]:
                l0, l1 = g0 - s * SUPER, g1 - s * SUPER
                nc.scalar.activation(
                    out=sc[:, l0:l1],
                    in_=ps[:, l0:l1],
                    func=mybir.ActivationFunctionType.Exp,
                    bias=bias_sb[:, :],
                    scale=two_beta,
                    accum_out=acc[:, c * maxp + p:c * maxp + p + 1],
                )

        class_sum = singles.tile([128, C], f32)
        nc.vector.tensor_reduce(
            out=class_sum[:, :],
            in_=acc.rearrange("q (c m) -> q c m", c=C),
            axis=mybir.AxisListType.X,
            op=mybir.AluOpType.add,
        )
        logits_sb = singles.tile([128, C], f32)
        nc.scalar.activation(
            out=logits_sb[:, :],
            in_=class_sum[:, :],
            func=mybir.ActivationFunctionType.Ln,
            bias=eps_sb[:, :],
            scale=1.0,
        )
        nc.vector.tensor_scalar_mul(logits_sb[:, :], logits_sb[:, :], float(gamma))
        nc.sync.dma_start(out=out_d[:, :], in_=logits_sb[:, :])

    nc.compile()
    return nc


def _prepare(x, ex_feats, ex_labels, w_unconstrained, gamma_unconstrained,
             beta_unconstrained):
    from concourse import mybir

    x = np.asarray(x, dtype=np.float64)
    e = np.asarray(ex_feats, dtype=np.float64)
    labels = np.asarray(ex_labels).astype(np.int64)
    wu = np.asarray(w_unconstrained, dtype=np.float64)

    beta = float(np.log1p(np.exp(np.float64(beta_unconstrained)))) + EPS
    gamma = float(np.log1p(np.exp(np.float64(gamma_unconstrained)))) + EPS
    wexp = np.exp(wu - wu.max())
    w = wexp / wexp.sum() + EPS

    perm = np.argsort(labels, kind="stable")
    e_sorted = e[perm]
    counts = np.bincount(labels[perm], minlength=C)

    bf16 = _np_dt(mybir, "bfloat16")
    e_np = _np_dt(mybir, E_DTYPE)

    # e_t[kc, r, n] = e_sorted[n, kc*128 + r]
    e_t = np.ascontiguousarray(
        e_sorted.T.reshape(KC, 128, N)).astype(e_np)

    xw = x * w[None, :]                               # (B, D)
    x2w = (x * x) @ w                                 # (B,)
    e2w = (e_sorted * e_sorted) @ w                   # (N,)

    aug = np.zeros((1, N + 128), dtype=bf16)
    aug[0, :N] = (-0.5 * e2w).astype(bf16)
    aug[0, N:] = np.ones(128, dtype=bf16)

    per_core = []
    for cid in range(NCORES):
        rows = slice(cid * B_LOC, (cid + 1) * B_LOC)
        xw_c = xw[rows]                               # (128, 512)
        # xw_t[r, kc, m] = xw_c[m, kc*128+r]
        xw_t = np.ascontiguousarray(
            xw_c.T.reshape(KC, 128, B_LOC).transpose(1, 0, 2)).astype(bf16)
        bias_c = (-beta * x2w[rows]).astype(np.float32).reshape(B_LOC, 1)
        per_core.append({
            "e_t": e_t,
            "xw_t": xw_t,
            "aug": aug,
            "bias": bias_c,
        })
    return per_core, counts, beta, gamma


def kernel(x, ex_feats, ex_labels, w_unconstrained, gamma_unconstrained,
           beta_unconstrained, _want_results=False, **run_kwargs):
    from concourse.bass_utils import run_bass_kernel_spmd

    per_core, counts, beta, gamma = _prepare(
        x, ex_feats, ex_labels, w_unconstrained, gamma_unconstrained,
        beta_unconstrained)

    pieces, maxp = _compute_pieces(counts)
    key = (tuple(pieces), maxp, round(beta, 12), round(gamma, 12), E_DTYPE)
    if key not in _prog_cache:
        _prog_cache[key] = _build_program(pieces, maxp, beta, gamma)
    nc = _prog_cache[key]

    res = run_bass_kernel_spmd(nc, per_core, list(range(NCORES)), **run_kwargs)
    out = np.concatenate(
        [np.asarray(res.results[cid]["logits"], dtype=np.float32)
         for cid in range(NCORES)], axis=0)
    if _want_results:
        return out, res
    return out


# revision 9
# speedup vs baseline: 1.0321x; 1.0321x over previous
"""ExemplarAttention Trainium2 kernel (8 NeuronCores, batch-sharded).

logits[b,c] = gamma * log(sum_{n:label[n]=c} exp(-beta * sum_k w_k (x[b,k]-e[n,k])^2) + eps)

Strategy:
  - Data-parallel over batch B=1024 across 8 cores (128 rows/core = one partition tile).
  - Host precomputes the tiny constrained params (softmax(w), beta, gamma),
    x^2@w (per-row bias), e^2@w, and sorts exemplars by class label so the
    per-class scatter-add becomes contiguous segment sums.
  - On device, per 2048-column PSUM super-tile:
      psum = (ones x -S*e2w/2)            [K=1 bf16 matmul, start=True]
           + S * sum_k xw_t[k].T @ e_t[k] [fp8 DoubleRow matmuls, 2 groups x K=256]
    i.e. psum[m,n] = S * (cross[m,n] - e2w[n]/2).   (S rescales x*w into fp8 range)
  - ScalarE: exp((2*beta/S)*psum + (-beta*x2w)[m]) per class-segment piece with
    accum_out -> per-class partial sums directly (no one-hot GEMM, no transpose).
  - Tail: one 3D tensor_reduce merges the piece partials, Ln(+1e-9), *gamma, DMA out.
"""

import os
from contextlib import ExitStack

import numpy as np

B, N, D, C = 1024, 16384, 512, 10
NCORES = 8
B_LOC = B // NCORES          # 128
NG = 2                       # DoubleRow groups (K=256 each)
SUPER = 2048                 # psum super-tile width (4 banks)
NSUPER = N // SUPER
NTILE = 512                  # matmul free dim (1 psum bank)
EPS = 1e-9
S_SCALE = 128.0              # fp8 scale applied to x*w (and the e2w aug row)

# e_t DMA blocks: (col_start, width). First two supers are loaded alone so the
# first matmuls start sooner; the rest stream as 4096-wide transfers.
ET_BLOCKS = [(0, 2048), (2048, 2048), (4096, 4096), (8192, 4096), (12288, 4096)]

_prog_cache = {}


def _np_dt(mybir, name):
    return mybir.dt.np(getattr(mybir.dt, name))


def _compute_pieces(counts):
    """Split each class's sorted-exemplar segment at SUPER boundaries.

    Returns (pieces, maxp): pieces is a list of (super_idx, cls, piece_idx,
    g0, g1) with global column range [g0, g1)."""
    starts = np.concatenate([[0], np.cumsum(counts)]).astype(int)
    pieces = []
    piece_counter = [0] * C
    for c in range(C):
        g0, g1 = int(starts[c]), int(starts[c + 1])
        while g0 < g1:
            end = min(g1, (g0 // SUPER + 1) * SUPER)
            pieces.append((g0 // SUPER, c, piece_counter[c], g0, end))
            piece_counter[c] += 1
            g0 = end
    maxp = max(piece_counter) if max(piece_counter) > 0 else 1
    return pieces, maxp


def _build_program(pieces, maxp, beta, gamma):
    import concourse.bass as bass  # noqa: F401
    import concourse.tile as tile
    from concourse import bacc, mybir

    fp8 = mybir.dt.float8e4
    bf16 = mybir.dt.bfloat16
    f32 = mybir.dt.float32

    nc = bacc.Bacc("TRN2", target_bir_lowering=False, debug=False,
                   num_devices=NCORES)

    e_t_d = nc.dram_tensor("e_t", [NG, 128, 2, N], fp8, kind="ExternalInput").ap()
    xw_t_d = nc.dram_tensor("xw_t", [128, NG, 2, B_LOC], fp8,
                            kind="ExternalInput").ap()
    aug_d = nc.dram_tensor("aug", [1, N + 128], bf16, kind="ExternalInput").ap()
    bias_d = nc.dram_tensor("bias", [B_LOC, 1], f32, kind="ExternalInput").ap()
    out_d = nc.dram_tensor("logits", [B_LOC, C], f32, kind="ExternalOutput").ap()

    act_scale = float(2.0 * beta / S_SCALE)

    by_super = [[] for _ in range(NSUPER)]
    for s, c, p, g0, g1 in pieces:
        by_super[s].append((c, p, g0, g1))

    # super -> (block index, col offset within block)
    sup_block = {}
    for bi, (c0, w) in enumerate(ET_BLOCKS):
        for s in range(c0 // SUPER, (c0 + w) // SUPER):
            sup_block[s] = (bi, s * SUPER - c0)

    with tile.TileContext(nc) as tc, ExitStack() as ctx:
        singles = ctx.enter_context(tc.tile_pool(name="singles", bufs=1))
        et_pool = ctx.enter_context(tc.tile_pool(name="et", bufs=3 * NG))
        psum_pool = ctx.enter_context(tc.tile_pool(name="ps", bufs=2, space="PSUM"))
        sc_pool = ctx.enter_context(tc.tile_pool(name="sc", bufs=2))

        # Dummy activation first so the ACT table load runs during the DMA
        # startup window instead of blocking the first real exp.
        dummy = singles.tile([128, 1], f32)
        nc.vector.memset(dummy[:, :], 0.0)
        nc.scalar.activation(out=dummy[:, :], in_=dummy[:, :],
                             func=mybir.ActivationFunctionType.Exp, scale=1.0)

        # Small constant loads on gpsimd (SWDGE) to keep sync/vector free
        # for the exemplar stream.
        xw_sb = singles.tile([128, NG, 2, B_LOC], fp8)
        nc.gpsimd.dma_start(out=xw_sb[:, :, :, :], in_=xw_t_d[:, :, :, :])
        # aug row (-S*e2w/2) and the ones row for the K=1 matmul share one
        # tile so their base partitions match.
        aug_sb = singles.tile([1, N + 128], bf16)
        nc.gpsimd.dma_start(out=aug_sb[:, :], in_=aug_d[:, :])
        bias_sb = singles.tile([B_LOC, 1], f32)
        nc.gpsimd.dma_start(out=bias_sb[:, :], in_=bias_d[:, :])

        acc = singles.tile([128, C * maxp], f32)
        nc.vector.memset(acc[:, :], 0.0)
        eps_sb = singles.tile([128, 1], f32)
        nc.vector.memset(eps_sb[:, :], float(EPS))

        # e_t streaming: alternate dispatch between the sync and vector
        # sequencers (each DIRECT2D dispatch costs ~0.6us on its sequencer).
        et_tiles = {}
        dma_engines = [nc.sync, nc.scalar]
        di = 0
        for bi, (c0, w) in enumerate(ET_BLOCKS):
            for g in range(NG):
                et = et_pool.tile([128, 2, 4096], fp8, tag="et")
                dma_engines[di % 2].dma_start(
                    out=et[:, :, :w], in_=e_t_d[g, :, :, c0:c0 + w])
                di += 1
                et_tiles[(bi, g)] = et

        for s in range(NSUPER):
            bi, off = sup_block[s]
            ps = psum_pool.tile([128, SUPER], f32)
            # one K=1 aug matmul per bank (start=True fills the whole bank)
            for j in range(SUPER // NTILE):
                cs = slice(j * NTILE, (j + 1) * NTILE)
                gcs = slice(s * SUPER + j * NTILE, s * SUPER + (j + 1) * NTILE)
                nc.tensor.matmul(ps[:, cs], lhsT=aug_sb[:, N:N + B_LOC],
                                 rhs=aug_sb[:, gcs], start=True, stop=False)
            # DoubleRow main matmuls, k-major so weights reload once per group
            for g in range(NG):
                et = et_tiles[(bi, g)]
                for j in range(SUPER // NTILE):
                    cs = slice(j * NTILE, (j + 1) * NTILE)
                    ecs = slice(off + j * NTILE, off + (j + 1) * NTILE)
                    nc.tensor.matmul(
                        ps[:, cs], lhsT=xw_sb[:, g, :, :],
                        rhs=et[:, :, ecs], start=False, stop=(g == NG - 1),
                        perf_mode=mybir.MatmulPerfMode.DoubleRow)

            sc = sc_pool.tile([128, SUPER], f32, tag="sc")
            for c, p, g0, g1 in by_super[s]:
                l0, l1 = g0 - s * SUPER, g1 - s * SUPER
                nc.scalar.activation(
                    out=sc[:, l0:l1],
                    in_=ps[:, l0:l1],
                    func=mybir.ActivationFunctionType.Exp,
                    bias=bias_sb[:, :],
                    scale=act_scale,
                    accum_out=acc[:, c * maxp + p:c * maxp + p + 1],
                )

        class_sum = singles.tile([128, C], f32)
        nc.vector.tensor_reduce(
            out=class_sum[:, :],
            in_=acc.rearrange("q (c m) -> q c m", c=C),
            axis=mybir.AxisListType.X,
            op=mybir.AluOpType.add,
        )
        logits_sb = singles.tile([128, C], f32)
        nc.scalar.activation(
            out=logits_sb[:, :],
            in_=class_sum[:, :],
            func=mybir.ActivationFunctionType.Ln,
            bias=eps_sb[:, :],
            scale=1.0,
        )
        nc.vector.tensor_scalar_mul(logits_sb[:, :], logits_sb[:, :], float(gamma))
        nc.sync.dma_start(out=out_d[:, :], in_=logits_sb[:, :])

    nc.compile()

    # Both Exp and Ln live in act-func-set 6 (natural_log_exp_and_others);
    # the insertion pass picks per-func sets and pays a mid-kernel reload.
    # Point the first load at set 6 and drop the now-redundant extras.
    loads = [(b, i) for b in nc.main_func.blocks for i in b.instructions
             if isinstance(i, mybir.InstLoadActFuncSet)]
    if loads:
        loads[0][1].act_func_set_id = 6
        for b, i in loads[1:]:
            if i.sync_info is None or (
                    not i.sync_info.on_wait and not i.sync_info.on_update):
                b.instructions.remove(i)
            else:
                i.act_func_set_id = 6
    return nc


def _prepare(x, ex_feats, ex_labels, w_unconstrained, gamma_unconstrained,
             beta_unconstrained):
    from concourse import mybir

    x = np.asarray(x, dtype=np.float64)
    e = np.asarray(ex_feats, dtype=np.float64)
    labels = np.asarray(ex_labels).astype(np.int64)
    wu = np.asarray(w_unconstrained, dtype=np.float64)

    beta = float(np.log1p(np.exp(np.float64(beta_unconstrained)))) + EPS
    gamma = float(np.log1p(np.exp(np.float64(gamma_unconstrained)))) + EPS
    wexp = np.exp(wu - wu.max())
    w = wexp / wexp.sum() + EPS

    perm = np.argsort(labels, kind="stable")
    e_sorted = e[perm]
    counts = np.bincount(labels[perm], minlength=C)

    bf16 = _np_dt(mybir, "bfloat16")
    fp8 = _np_dt(mybir, "float8e4")

    # e_t[g, r, s, n] = e_sorted[n, (2g+s)*128 + r]
    e_t = np.ascontiguousarray(
        e_sorted.T.reshape(NG, 2, 128, N).transpose(0, 2, 1, 3)).astype(fp8)

    xw = x * w[None, :]                               # (B, D)
    x2w = (x * x) @ w                                 # (B,)
    e2w = (e_sorted * e_sorted) @ w                   # (N,)

    aug = np.zeros((1, N + 128), dtype=bf16)
    aug[0, :N] = (-0.5 * S_SCALE * e2w).astype(bf16)
    aug[0, N:] = np.ones(128, dtype=bf16)

    per_core = []
    for cid in range(NCORES):
        rows = slice(cid * B_LOC, (cid + 1) * B_LOC)
        xw_c = S_SCALE * xw[rows]                     # (128, 512)
        # xw_t[r, g, s, m] = S * xw_c[m, (2g+s)*128+r]
        xw_t = np.ascontiguousarray(
            xw_c.T.reshape(NG, 2, 128, B_LOC).transpose(2, 0, 1, 3)).astype(fp8)
        bias_c = (-beta * x2w[rows]).astype(np.float32).reshape(B_LOC, 1)
        per_core.append({
            "e_t": e_t,
            "xw_t": xw_t,
            "aug": aug,
            "bias": bias_c,
        })
    return per_core, counts, beta, gamma


def kernel(x, ex_feats, ex_labels, w_unconstrained, gamma_unconstrained,
           beta_unconstrained, _want_results=False, **run_kwargs):
    from concourse.bass_utils import run_bass_kernel_spmd

    per_core, counts, beta, gamma = _prepare(
        x, ex_feats, ex_labels, w_unconstrained, gamma_unconstrained,
        beta_unconstrained)

    pieces, maxp = _compute_pieces(counts)
    key = (tuple(pieces), maxp, round(beta, 12), round(gamma, 12))
    if key not in _prog_cache:
        _prog_cache[key] = _build_program(pieces, maxp, beta, gamma)
    nc = _prog_cache[key]

    res = run_bass_kernel_spmd(nc, per_core, list(range(NCORES)), **run_kwargs)
    out = np.concatenate(
        [np.asarray(res.results[cid]["logits"], dtype=np.float32)
         for cid in range(NCORES)], axis=0)
    if _want_results:
        return out, res
    return out


# revision 11
# speedup vs baseline: 1.0810x; 1.0473x over previous
"""ExemplarAttention Trainium2 kernel (8 NeuronCores, batch-sharded).

logits[b,c] = gamma * log(sum_{n:label[n]=c} exp(-beta * sum_k w_k (x[b,k]-e[n,k])^2) + eps)

Strategy:
  - Data-parallel over batch B=1024 across 8 cores (128 rows/core = one partition tile).
  - Host precomputes the tiny constrained params (softmax(w), beta, gamma),
    x^2@w (per-row bias), e^2@w, and sorts exemplars by class label so the
    per-class scatter-add becomes contiguous segment sums.
  - On device, per 2048-column PSUM super-tile:
      psum = (ones x -S*e2w/2)            [K=1 bf16 matmul, start=True]
           + S * sum_k xw_t[k].T @ e_t[k] [fp8 DoubleRow matmuls, 2 groups x K=256]
    i.e. psum[m,n] = S * (cross[m,n] - e2w[n]/2).   (S rescales x*w into fp8 range)
  - ScalarE: exp((2*beta/S)*psum + (-beta*x2w)[m]) per class-segment piece with
    accum_out -> per-class partial sums directly (no one-hot GEMM, no transpose).
  - Tail: one 3D tensor_reduce merges the piece partials, Ln(+1e-9), *gamma, DMA out.
"""

import os
from contextlib import ExitStack

import numpy as np

B, N, D, C = 1024, 16384, 512, 10
NCORES = 8
B_LOC = B // NCORES          # 128
NG = 2                       # DoubleRow groups (K=256 each)
SUPER = 2048                 # psum super-tile width (4 banks)
NSUPER = N // SUPER
NTILE = 512                  # matmul free dim (1 psum bank)
EPS = 1e-9
S_SCALE = 128.0              # fp8 scale applied to x*w (and the e2w aug row)

# e_t DMA blocks: (col_start, width). First two supers are loaded alone so the
# first matmuls start sooner; the rest stream as 4096-wide transfers.
ET_BLOCKS = [(0, 2048), (2048, 2048), (4096, 4096), (8192, 4096), (12288, 4096)]

_prog_cache = {}


def _np_dt(mybir, name):
    return mybir.dt.np(getattr(mybir.dt, name))


def _compute_pieces(counts):
    """Split each class's sorted-exemplar segment at SUPER boundaries.

    Returns (pieces, maxp): pieces is a list of (super_idx, cls, piece_idx,
    g0, g1) with global column range [g0, g1)."""
    starts = np.concatenate([[0], np.cumsum(counts)]).astype(int)
    pieces = []
    piece_counter = [0] * C
    for c in range(C):
        g0, g1 = int(starts[c]), int(starts[c + 1])
        while g0 < g1:
            end = min(g1, (g0 // SUPER + 1) * SUPER)
            pieces.append((g0 // SUPER, c, piece_counter[c], g0, end))
            piece_counter[c] += 1
            g0 = end
    maxp = max(piece_counter) if max(piece_counter) > 0 else 1
    return pieces, maxp


def _build_program(pieces, maxp, beta, gamma):
    import concourse.bass as bass  # noqa: F401
    import concourse.tile as tile
    from concourse import bacc, mybir

    fp8 = mybir.dt.float8e4
    bf16 = mybir.dt.bfloat16
    f32 = mybir.dt.float32

    nc = bacc.Bacc("TRN2", target_bir_lowering=False, debug=False,
                   num_devices=NCORES)

    e_t_d = nc.dram_tensor("e_t", [NG, 128, 2, N], fp8, kind="ExternalInput").ap()
    xw_t_d = nc.dram_tensor("xw_t", [128, NG, 2, B_LOC], fp8,
                            kind="ExternalInput").ap()
    aug_d = nc.dram_tensor("aug", [1, N + 128], bf16, kind="ExternalInput").ap()
    bias_d = nc.dram_tensor("bias", [B_LOC, 1], f32, kind="ExternalInput").ap()
    out_d = nc.dram_tensor("logits", [B_LOC, C], f32, kind="ExternalOutput").ap()

    act_scale = float(2.0 * beta / S_SCALE)

    by_super = [[] for _ in range(NSUPER)]
    for s, c, p, g0, g1 in pieces:
        by_super[s].append((c, p, g0, g1))

    # super -> (block index, col offset within block)
    sup_block = {}
    for bi, (c0, w) in enumerate(ET_BLOCKS):
        for s in range(c0 // SUPER, (c0 + w) // SUPER):
            sup_block[s] = (bi, s * SUPER - c0)

    with tile.TileContext(nc) as tc, ExitStack() as ctx:
        singles = ctx.enter_context(tc.tile_pool(name="singles", bufs=1))
        et_pool = ctx.enter_context(tc.tile_pool(name="et", bufs=len(ET_BLOCKS) * NG))
        psum_pool = ctx.enter_context(tc.tile_pool(name="ps", bufs=2, space="PSUM"))
        sc_pool = ctx.enter_context(tc.tile_pool(name="sc", bufs=2))

        # Dummy activation first so the ACT table load runs during the DMA
        # startup window instead of blocking the first real exp.
        dummy = singles.tile([128, 1], f32)
        nc.vector.memset(dummy[:, :], 0.0)
        nc.scalar.activation(out=dummy[:, :], in_=dummy[:, :],
                             func=mybir.ActivationFunctionType.Exp, scale=1.0)

        # e_t streaming: alternate dispatch between the sync and scalar
        # sequencers (each DIRECT2D dispatch costs ~0.6us on its sequencer).
        # Block 0 is dispatched before everything else so its lines reach
        # the DMA queues first and the first matmuls start ASAP.
        et_tiles = {}
        dma_engines = [nc.sync, nc.scalar]
        di = 0
        for bi, (c0, w) in enumerate(ET_BLOCKS):
            for g in range(NG):
                et = et_pool.tile([128, 2, 4096], fp8, tag="et")
                et_tiles[(bi, g)] = et

        def load_et(bi, g):
            nonlocal di
            c0, w = ET_BLOCKS[bi]
            dma_engines[di % 2].dma_start(
                out=et_tiles[(bi, g)][:, :, :w], in_=e_t_d[g, :, :, c0:c0 + w])
            di += 1

        load_et(0, 0)
        load_et(0, 1)

        # Small constant loads on gpsimd (SWDGE) to keep sync/scalar free
        # for the exemplar stream.
        xw_sb = singles.tile([128, NG, 2, B_LOC], fp8)
        nc.gpsimd.dma_start(out=xw_sb[:, :, :, :], in_=xw_t_d[:, :, :, :])
        # aug row (-S*e2w/2) and the ones row for the K=1 matmul share one
        # tile so their base partitions match.
        aug_sb = singles.tile([1, N + 128], bf16)
        nc.gpsimd.dma_start(out=aug_sb[:, :], in_=aug_d[:, :])
        bias_sb = singles.tile([B_LOC, 1], f32)
        nc.gpsimd.dma_start(out=bias_sb[:, :], in_=bias_d[:, :])

        acc = singles.tile([128, C * maxp], f32)
        nc.vector.memset(acc[:, :], 0.0)
        eps_sb = singles.tile([128, 1], f32)
        nc.vector.memset(eps_sb[:, :], float(EPS))

        for bi in range(1, len(ET_BLOCKS)):
            for g in range(NG):
                load_et(bi, g)

        for s in range(NSUPER):
            bi, off = sup_block[s]
            ps = psum_pool.tile([128, SUPER], f32)
            # one K=1 aug matmul per bank (start=True fills the whole bank)
            for j in range(SUPER // NTILE):
                cs = slice(j * NTILE, (j + 1) * NTILE)
                gcs = slice(s * SUPER + j * NTILE, s * SUPER + (j + 1) * NTILE)
                nc.tensor.matmul(ps[:, cs], lhsT=aug_sb[:, N:N + B_LOC],
                                 rhs=aug_sb[:, gcs], start=True, stop=False)
            # DoubleRow main matmuls, k-major so weights reload once per group
            for g in range(NG):
                et = et_tiles[(bi, g)]
                for j in range(SUPER // NTILE):
                    cs = slice(j * NTILE, (j + 1) * NTILE)
                    ecs = slice(off + j * NTILE, off + (j + 1) * NTILE)
                    nc.tensor.matmul(
                        ps[:, cs], lhsT=xw_sb[:, g, :, :],
                        rhs=et[:, :, ecs], start=False, stop=(g == NG - 1),
                        perf_mode=mybir.MatmulPerfMode.DoubleRow)

            sc = sc_pool.tile([128, SUPER], f32, tag="sc")
            for c, p, g0, g1 in by_super[s]:
                l0, l1 = g0 - s * SUPER, g1 - s * SUPER
                nc.scalar.activation(
                    out=sc[:, l0:l1],
                    in_=ps[:, l0:l1],
                    func=mybir.ActivationFunctionType.Exp,
                    bias=bias_sb[:, :],
                    scale=act_scale,
                    accum_out=acc[:, c * maxp + p:c * maxp + p + 1],
                )

        class_sum = singles.tile([128, C], f32)
        nc.vector.tensor_reduce(
            out=class_sum[:, :],
            in_=acc.rearrange("q (c m) -> q c m", c=C),
            axis=mybir.AxisListType.X,
            op=mybir.AluOpType.add,
        )
        logits_sb = singles.tile([128, C], f32)
        nc.scalar.activation(
            out=logits_sb[:, :],
            in_=class_sum[:, :],
            func=mybir.ActivationFunctionType.Ln,
            bias=eps_sb[:, :],
            scale=1.0,
        )
        nc.vector.tensor_scalar_mul(logits_sb[:, :], logits_sb[:, :], float(gamma))
        nc.sync.dma_start(out=out_d[:, :], in_=logits_sb[:, :])

    nc.compile()

    # Both Exp and Ln live in act-func-set 6 (natural_log_exp_and_others);
    # the insertion pass picks per-func sets and pays a mid-kernel reload.
    # Point the first load at set 6 and drop the now-redundant extras.
    loads = [(b, i) for b in nc.main_func.blocks for i in b.instructions
             if isinstance(i, mybir.InstLoadActFuncSet)]
    if loads:
        loads[0][1].act_func_set_id = 6
        for b, i in loads[1:]:
            if i.sync_info is None or (
                    not i.sync_info.on_wait and not i.sync_info.on_update):
                b.instructions.remove(i)
            else:
                i.act_func_set_id = 6
    return nc


def _prepare(x, ex_feats, ex_labels, w_unconstrained, gamma_unconstrained,
             beta_unconstrained):
    from concourse import mybir

    x = np.asarray(x, dtype=np.float64)
    e = np.asarray(ex_feats, dtype=np.float64)
    labels = np.asarray(ex_labels).astype(np.int64)
    wu = np.asarray(w_unconstrained, dtype=np.float64)

    beta = float(np.log1p(np.exp(np.float64(beta_unconstrained)))) + EPS
    gamma = float(np.log1p(np.exp(np.float64(gamma_unconstrained)))) + EPS
    wexp = np.exp(wu - wu.max())
    w = wexp / wexp.sum() + EPS

    perm = np.argsort(labels, kind="stable")
    e_sorted = e[perm]
    counts = np.bincount(labels[perm], minlength=C)

    bf16 = _np_dt(mybir, "bfloat16")
    fp8 = _np_dt(mybir, "float8e4")

    # e_t[g, r, s, n] = e_sorted[n, (2g+s)*128 + r]
    e_t = np.ascontiguousarray(
        e_sorted.T.reshape(NG, 2, 128, N).transpose(0, 2, 1, 3)).astype(fp8)

    xw = x * w[None, :]                               # (B, D)
    x2w = (x * x) @ w                                 # (B,)
    e2w = (e_sorted * e_sorted) @ w                   # (N,)

    aug = np.zeros((1, N + 128), dtype=bf16)
    aug[0, :N] = (-0.5 * S_SCALE * e2w).astype(bf16)
    aug[0, N:] = np.ones(128, dtype=bf16)

    per_core = []
    for cid in range(NCORES):
        rows = slice(cid * B_LOC, (cid + 1) * B_LOC)
        xw_c = S_SCALE * xw[rows]                     # (128, 512)
        # xw_t[r, g, s, m] = S * xw_c[m, (2g+s)*128+r]
        xw_t = np.ascontiguousarray(
            xw_c.T.reshape(NG, 2, 128, B_LOC).transpose(2, 0, 1, 3)).astype(fp8)
        bias_c = (-beta * x2w[rows]).astype(np.float32).reshape(B_LOC, 1)
        per_core.append({
            "e_t": e_t,
            "xw_t": xw_t,
            "aug": aug,
            "bias": bias_c,
        })
    return per_core, counts, beta, gamma


def kernel(x, ex_feats, ex_labels, w_unconstrained, gamma_unconstrained,
           beta_unconstrained, _want_results=False, **run_kwargs):
    from concourse.bass_utils import run_bass_kernel_spmd

    per_core, counts, beta, gamma = _prepare(
        x, ex_feats, ex_labels, w_unconstrained, gamma_unconstrained,
        beta_unconstrained)

    pieces, maxp = _compute_pieces(counts)
    key = (tuple(pieces), maxp, round(beta, 12), round(gamma, 12))
    if key not in _prog_cache:
        _prog_cache[key] = _build_program(pieces, maxp, beta, gamma)
    nc = _prog_cache[key]

    res = run_bass_kernel_spmd(nc, per_core, list(range(NCORES)), **run_kwargs)
    out = np.concatenate(
        [np.asarray(res.results[cid]["logits"], dtype=np.float32)
         for cid in range(NCORES)], axis=0)
    if _want_results:
        return out, res
    return out


# revision 15
# speedup vs baseline: 1.1617x; 1.0747x over previous
"""ExemplarAttention Trainium2 kernel (8 NeuronCores, batch-sharded).

logits[b,c] = gamma * log(sum_{n:label[n]=c} exp(-beta * sum_k w_k (x[b,k]-e[n,k])^2) + eps)

Strategy:
  - Data-parallel over batch B=1024 across 8 cores (128 rows/core = one partition tile).
  - Host precomputes the tiny constrained params (softmax(w), beta, gamma),
    x^2@w (per-row bias), e^2@w, and sorts exemplars by class label so the
    per-class scatter-add becomes contiguous segment sums.
  - On device, per 2048-column PSUM super-tile:
      psum = (ones x -S*e2w/2)            [K=1 bf16 matmul, start=True]
           + S * sum_k xw_t[k].T @ e_t[k] [fp8 DoubleRow matmuls, 2 groups x K=256]
    i.e. psum[m,n] = S * (cross[m,n] - e2w[n]/2).   (S rescales x*w into fp8 range)
  - ScalarE: exp((2*beta/S)*psum + (-beta*x2w)[m]) per class-segment piece with
    accum_out -> per-class partial sums directly (no one-hot GEMM, no transpose).
  - Tail: one 3D tensor_reduce merges the piece partials, Ln(+1e-9), *gamma, DMA out.
"""

import os
from contextlib import ExitStack

import numpy as np

B, N, D, C = 1024, 16384, 512, 10
NCORES = 8
B_LOC = B // NCORES          # 128
NG = 2                       # DoubleRow groups (K=256 each)
SUPER = 2048                 # psum super-tile width (4 banks)
NSUPER = N // SUPER
NTILE = 512                  # matmul free dim (1 psum bank)
EPS = 1e-9
S_SCALE = 128.0              # fp8 scale applied to x*w (and the e2w aug row)

# e_t DMA blocks: (col_start, width). First two supers are loaded alone so the
# first matmuls start sooner; the rest stream as 4096-wide transfers.
ET_BLOCKS = [(0, 2048), (2048, 2048), (4096, 4096), (8192, 4096), (12288, 4096)]

_prog_cache = {}


def _np_dt(mybir, name):
    return mybir.dt.np(getattr(mybir.dt, name))


def _compute_pieces(counts):
    """Split each class's sorted-exemplar segment at SUPER boundaries.

    Returns (pieces, maxp): pieces is a list of (super_idx, cls, piece_idx,
    g0, g1) with global column range [g0, g1)."""
    starts = np.concatenate([[0], np.cumsum(counts)]).astype(int)
    pieces = []
    piece_counter = [0] * C
    for c in range(C):
        g0, g1 = int(starts[c]), int(starts[c + 1])
        while g0 < g1:
            end = min(g1, (g0 // SUPER + 1) * SUPER)
            pieces.append((g0 // SUPER, c, piece_counter[c], g0, end))
            piece_counter[c] += 1
            g0 = end
    maxp = max(piece_counter) if max(piece_counter) > 0 else 1
    return pieces, maxp


def _build_program(pieces, maxp, beta, gamma):
    import concourse.bass as bass  # noqa: F401
    import concourse.tile as tile
    from concourse import bacc, mybir

    fp8 = mybir.dt.float8e4
    bf16 = mybir.dt.bfloat16
    f32 = mybir.dt.float32

    nc = bacc.Bacc("TRN2", target_bir_lowering=False, debug=False,
                   num_devices=NCORES)

    e_t_d = nc.dram_tensor("e_t", [NG, 128, 2, N], fp8, kind="ExternalInput").ap()
    xw_t_d = nc.dram_tensor("xw_t", [128, NG, 2, B_LOC], fp8,
                            kind="ExternalInput").ap()
    aug_d = nc.dram_tensor("aug", [1, N + 128], bf16, kind="ExternalInput").ap()
    bias_d = nc.dram_tensor("bias", [B_LOC, 1], f32, kind="ExternalInput").ap()
    out_d = nc.dram_tensor("logits", [B_LOC, C], f32, kind="ExternalOutput").ap()

    act_scale = float(2.0 * beta / S_SCALE)

    by_super = [[] for _ in range(NSUPER)]
    for s, c, p, g0, g1 in pieces:
        by_super[s].append((c, p, g0, g1))

    # super -> (block index, col offset within block)
    sup_block = {}
    for bi, (c0, w) in enumerate(ET_BLOCKS):
        for s in range(c0 // SUPER, (c0 + w) // SUPER):
            sup_block[s] = (bi, s * SUPER - c0)

    with tile.TileContext(nc) as tc, ExitStack() as ctx:
        singles = ctx.enter_context(tc.tile_pool(name="singles", bufs=1))
        et_pool = ctx.enter_context(tc.tile_pool(name="et", bufs=len(ET_BLOCKS) * NG))
        psum_pool = ctx.enter_context(tc.tile_pool(name="ps", bufs=2, space="PSUM"))
        sc_pool = ctx.enter_context(tc.tile_pool(name="sc", bufs=2))

        # Dummy activation first so the ACT table load runs during the DMA
        # startup window instead of blocking the first real exp.
        dummy = singles.tile([128, 1], f32)
        nc.vector.memset(dummy[:, :], 0.0)
        nc.scalar.activation(out=dummy[:, :], in_=dummy[:, :],
                             func=mybir.ActivationFunctionType.Exp, scale=1.0)

        # aug row (-S*e2w/2) + ones row, and the per-row exp bias: tiny, and
        # they gate the early aug matmuls -> head of the two HWDGE rings.
        aug_sb = singles.tile([1, N + 128], bf16)
        nc.sync.dma_start(out=aug_sb[:, :], in_=aug_d[:, :])
        bias_sb = singles.tile([B_LOC, 1], f32)
        nc.scalar.dma_start(out=bias_sb[:, :], in_=bias_d[:, :])

        et_tiles = {}
        dma_engines = [nc.sync, nc.scalar]
        di = 0
        for bi, (c0, w) in enumerate(ET_BLOCKS):
            for g in range(NG):
                et_tiles[(bi, g)] = et_pool.tile(
                    [128, 2, 4096], fp8, tag="et", name=f"et{bi}_{g}")

        def load_et(bi, g, eng=None):
            nonlocal di
            c0, w = ET_BLOCKS[bi]
            (eng or dma_engines[di % 2]).dma_start(
                out=et_tiles[(bi, g)][:, :, :w], in_=e_t_d[g, :, :, c0:c0 + w])
            di += 1

        # Block 0 rides at the head of both rings so super 0's matmuls can
        # start as soon as possible.
        load_et(0, 0, nc.sync)
        load_et(0, 1, nc.scalar)

        def emit_aug(s, ps):
            for j in range(SUPER // NTILE):
                cs = slice(j * NTILE, (j + 1) * NTILE)
                gcs = slice(s * SUPER + j * NTILE, s * SUPER + (j + 1) * NTILE)
                nc.tensor.matmul(ps[:, cs], lhsT=aug_sb[:, N:N + B_LOC],
                                 rhs=aug_sb[:, gcs], start=True, stop=False)

        # Hoist supers 0/1's aug matmuls: they only need aug_sb, so they run
        # during the e_t DMA window — prefilling PSUM and warming the PE
        # clock gate (HAM) before the DoubleRow stream lands.
        ps_pre = [psum_pool.tile([128, SUPER], f32, tag="ps", name=f"ps{s}")
                  for s in range(2)]
        for s in (0, 1):
            emit_aug(s, ps_pre[s])

        # x*w weights on gpsimd (SWDGE) to keep the HWDGE rings for e_t.
        xw_sb = singles.tile([128, NG, 2, B_LOC], fp8)
        nc.gpsimd.dma_start(out=xw_sb[:, :, :, :], in_=xw_t_d[:, :, :, :])

        acc = singles.tile([128, C * maxp], f32)
        nc.vector.memset(acc[:, :], 0.0)
        eps_sb = singles.tile([128, 1], f32)
        nc.vector.memset(eps_sb[:, :], float(EPS))

        for bi in range(1, len(ET_BLOCKS)):
            for g in range(NG):
                load_et(bi, g)

        for s in range(NSUPER):
            bi, off = sup_block[s]
            if s < 2:
                ps = ps_pre[s]
            else:
                ps = psum_pool.tile([128, SUPER], f32, tag="ps", name=f"ps{s}")
                emit_aug(s, ps)
            # DoubleRow main matmuls, k-major so weights reload once per group
            for g in range(NG):
                et = et_tiles[(bi, g)]
                for j in range(SUPER // NTILE):
                    cs = slice(j * NTILE, (j + 1) * NTILE)
                    ecs = slice(off + j * NTILE, off + (j + 1) * NTILE)
                    nc.tensor.matmul(
                        ps[:, cs], lhsT=xw_sb[:, g, :, :],
                        rhs=et[:, :, ecs], start=False, stop=(g == NG - 1),
                        perf_mode=mybir.MatmulPerfMode.DoubleRow)

            sc = sc_pool.tile([128, SUPER], f32, tag="sc")
            for c, p, g0, g1 in by_super[s]:
                l0, l1 = g0 - s * SUPER, g1 - s * SUPER
                nc.scalar.activation(
                    out=sc[:, l0:l1],
                    in_=ps[:, l0:l1],
                    func=mybir.ActivationFunctionType.Exp,
                    bias=bias_sb[:, :],
                    scale=act_scale,
                    accum_out=acc[:, c * maxp + p:c * maxp + p + 1],
                )

        class_sum = singles.tile([128, C], f32)
        nc.vector.tensor_reduce(
            out=class_sum[:, :],
            in_=acc.rearrange("q (c m) -> q c m", c=C),
            axis=mybir.AxisListType.X,
            op=mybir.AluOpType.add,
        )
        logits_sb = singles.tile([128, C], f32)
        nc.scalar.activation(
            out=logits_sb[:, :],
            in_=class_sum[:, :],
            func=mybir.ActivationFunctionType.Ln,
            bias=eps_sb[:, :],
            scale=1.0,
        )
        nc.vector.tensor_scalar_mul(logits_sb[:, :], logits_sb[:, :], float(gamma))
        nc.sync.dma_start(out=out_d[:, :], in_=logits_sb[:, :])

    nc.compile()

    # Both Exp and Ln live in act-func-set 6 (natural_log_exp_and_others);
    # the insertion pass picks per-func sets and pays a mid-kernel reload.
    # Point the first load at set 6 and drop the now-redundant extras.
    loads = [(b, i) for b in nc.main_func.blocks for i in b.instructions
             if isinstance(i, mybir.InstLoadActFuncSet)]
    if loads:
        loads[0][1].act_func_set_id = 6
        for b, i in loads[1:]:
            if i.sync_info is None or (
                    not i.sync_info.on_wait and not i.sync_info.on_update):
                b.instructions.remove(i)
            else:
                i.act_func_set_id = 6
    return nc


def _prepare(x, ex_feats, ex_labels, w_unconstrained, gamma_unconstrained,
             beta_unconstrained):
    from concourse import mybir

    x = np.asarray(x, dtype=np.float64)
    e = np.asarray(ex_feats, dtype=np.float64)
    labels = np.asarray(ex_labels).astype(np.int64)
    wu = np.asarray(w_unconstrained, dtype=np.float64)

    beta = float(np.log1p(np.exp(np.float64(beta_unconstrained)))) + EPS
    gamma = float(np.log1p(np.exp(np.float64(gamma_unconstrained)))) + EPS
    wexp = np.exp(wu - wu.max())
    w = wexp / wexp.sum() + EPS

    perm = np.argsort(labels, kind="stable")
    e_sorted = e[perm]
    counts = np.bincount(labels[perm], minlength=C)

    bf16 = _np_dt(mybir, "bfloat16")
    fp8 = _np_dt(mybir, "float8e4")

    # e_t[g, r, s, n] = e_sorted[n, (2g+s)*128 + r]
    e_t = np.ascontiguousarray(
        e_sorted.T.reshape(NG, 2, 128, N).transpose(0, 2, 1, 3)).astype(fp8)

    xw = x * w[None, :]                               # (B, D)
    x2w = (x * x) @ w                                 # (B,)
    e2w = (e_sorted * e_sorted) @ w                   # (N,)

    aug = np.zeros((1, N + 128), dtype=bf16)
    aug[0, :N] = (-0.5 * S_SCALE * e2w).astype(bf16)
    aug[0, N:] = np.ones(128, dtype=bf16)

    per_core = []
    for cid in range(NCORES):
        rows = slice(cid * B_LOC, (cid + 1) * B_LOC)
        xw_c = S_SCALE * xw[rows]                     # (128, 512)
        # xw_t[r, g, s, m] = S * xw_c[m, (2g+s)*128+r]
        xw_t = np.ascontiguousarray(
            xw_c.T.reshape(NG, 2, 128, B_LOC).transpose(2, 0, 1, 3)).astype(fp8)
        bias_c = (-beta * x2w[rows]).astype(np.float32).reshape(B_LOC, 1)
        per_core.append({
            "e_t": e_t,
            "xw_t": xw_t,
            "aug": aug,
            "bias": bias_c,
        })
    return per_core, counts, beta, gamma


def kernel(x, ex_feats, ex_labels, w_unconstrained, gamma_unconstrained,
           beta_unconstrained, _want_results=False, **run_kwargs):
    from concourse.bass_utils import run_bass_kernel_spmd

    per_core, counts, beta, gamma = _prepare(
        x, ex_feats, ex_labels, w_unconstrained, gamma_unconstrained,
        beta_unconstrained)

    pieces, maxp = _compute_pieces(counts)
    key = (tuple(pieces), maxp, round(beta, 12), round(gamma, 12))
    if key not in _prog_cache:
        _prog_cache[key] = _build_program(pieces, maxp, beta, gamma)
    nc = _prog_cache[key]

    res = run_bass_kernel_spmd(nc, per_core, list(range(NCORES)), **run_kwargs)
    out = np.concatenate(
        [np.asarray(res.results[cid]["logits"], dtype=np.float32)
         for cid in range(NCORES)], axis=0)
    if _want_results:
        return out, res
    return out
